# revision 2
# baseline (speedup 1.0000x reference)
"""GCN layer (N=8192, Cin=Cout=32) on 8 Trainium2 NeuronCores — v3 (fp8).

Math (matches the PyG-style reference):
    A = 2*adj off-diagonal, 1 on the diagonal
    deg[j]   = sum_i A[i,j] = 2*(colsum_j(adj) - adj[j,j]) + 1
    dis      = deg ** -0.5
    y        = x @ W
    z        = dis[:,None] * y
    s[j,:]   = 2*(adj^T z)[j,:] + (1-2*adj[j,j])*z[j,:]
    out      = tanh(dis[:,None]*s + b).T          # [32, 8192]

v3 vs v2:
  - adj is cast to fp8 e4m3 on the HOST (0/1 exact), halving the HBM
    stream to 8 MB/core (16 chunks x 512KB, 4KB/partition writes).
  - All slab matmuls (column sums + z^T@slab) run in fp8 DoubleRow perf
    mode (2 k-tiles per pass via the [128, 2, N] slab layout).  z is
    quantized to e4m3 pre-scaled by 128: the AllGather carries
    rdegS = 128^2/deg, consumers sqrt it into 128*dis, and the dis
    broadcast divides the scale back out.
  - DoubleRow is ISA-valid only at tile_position (0,0), so all DR
    outputs live on PSUM partitions 0:32 ("flat" layout); concurrent
    accumulation groups are separated by PSUM bank (pending-zero on
    start=True is 2KB-bank-granular).
  - The measured AllGather latency here is ~30us, so the pipeline is
    laid out to keep every engine queue free of cross-piece AG
    cascades: the DVE queue holds only local work (deg math, y copies,
    epilogue), the ACT queue holds all AG consumers (gather loads,
    sqrt, z scaling), and phase-2 is emitted piece-major AFTER the
    whole column-sum backlog so the PE never blocks an upcoming AG.
"""

import os

import numpy as np
import ml_dtypes

import concourse.bass as bass
import concourse.bacc as bacc
import concourse.mybir as mybir
import concourse.tile as tile
from concourse.bass_utils import run_bass_kernel_spmd

F32 = mybir.dt.float32
BF16 = mybir.dt.bfloat16
F8 = mybir.dt.float8e4
AF = mybir.ActivationFunctionType
Alu = mybir.AluOpType
PM = mybir.MatmulPerfMode

N = 8192          # nodes
C = 32            # channels (Cin == Cout)
NCORES = 8
JW = N // NCORES  # column-shard width per core (1024)
P = 128           # SBUF partitions
NT = N // P       # i-tiles (64)
NPC = 4           # column pieces (one AllGather each)
PW = JW // NPC    # piece width (256 cols)
TPC = 16          # i-tiles per slab DMA chunk (512KB fp8)
NCH = NT // TPC   # chunks per piece (4)
YR = 16           # y tiles per PSUM-scratch round
ZS = 128.0        # z fp8 pre-scale (power of 2; divided out in epilogue)

FILL = int(os.environ.get("FILL", "0"))
STEP_NS = float(os.environ.get("STEP_NS", "600"))
TAILFILL = float(os.environ.get("TAILFILL", "0"))
COL_NS = 0.4167   # ns per moving column at 2.4GHz

NP_F8 = ml_dtypes.float8_e4m3
BF = ml_dtypes.bfloat16


def build_kernel(n_devices=NCORES, repeat=1, serialize=False, variant="full"):
    nc = bacc.Bacc(
        "TRN2", target_bir_lowering=False, debug=False, num_devices=n_devices
    )

    adjs = nc.dram_tensor(
        "adjs", [NPC, NCH, P, TPC, PW], F8, kind="ExternalInput"
    ).ap()
    xT = nc.dram_tensor("xT", [C, N], BF16, kind="ExternalInput").ap()
    xTo = nc.dram_tensor("xTo", [C, JW], BF16, kind="ExternalInput").ap()
    wbf_d = nc.dram_tensor("Wbf", [C, C], BF16, kind="ExternalInput").ap()
    b_d = nc.dram_tensor("b", [C], F32, kind="ExternalInput").ap()
    adiag = nc.dram_tensor("adiag", [JW], F32, kind="ExternalInput").ap()
    out_d = nc.dram_tensor("out", [C, JW], F32, kind="ExternalOutput").ap()

    with tile.TileContext(nc) as tc:
        prev = None
        for _ in range(repeat):
            prev = _body(
                tc, adjs, xT, xTo, wbf_d, b_d, adiag, out_d, n_devices,
                prev_inst=prev if serialize else None,
                variant=variant,
            )

    nc.compile()
    return nc


def piece_of(t):
    return (t % (2 * NPC)) // 2


def _body(tc, adjs, xT, xTo, wbf_d, b_d, adiag, out_d, n_devices=NCORES,
          prev_inst=None, variant="full"):
    nc = tc.nc
    use_cc = n_devices > 1 and variant != "nocc"

    with (
        tc.tile_pool(name="const", bufs=1) as constp,
        tc.tile_pool(name="slabp", bufs=1) as slabp,
        tc.tile_pool(name="yp", bufs=1) as yp,
        tc.tile_pool(name="zp", bufs=1) as zp,
        tc.tile_pool(name="rowsp", bufs=1) as rowsp,
        tc.tile_pool(name="bcp", bufs=1) as bcp,
        tc.tile_pool(name="outp", bufs=1) as outp,
        tc.tile_pool(name="psyr", bufs=1, space="PSUM") as psyr,   # 1 bank
        tc.tile_pool(name="pscs", bufs=1, space="PSUM") as pscs,   # 1 bank
        tc.tile_pool(name="pss", bufs=1, space="PSUM") as pss,     # 4 banks
        tc.tile_pool(name="psbc", bufs=1, space="PSUM") as psbc,   # 1 bank
        tc.tile_pool(name="psyt", bufs=1, space="PSUM") as psyt,   # 1 bank
        tc.tile_pool(name="dramp", bufs=1, space="DRAM") as dramp,
    ):
        # ---- small inputs on the Pool DGE queue (sync queue = slab only) --
        w_bf = constp.tile([C, C], BF16)
        d0 = nc.gpsimd.dma_start(w_bf[:], wbf_d)
        if prev_inst is not None:
            from concourse.tile_rust import add_dep_helper
            add_dep_helper(d0.ins, prev_inst.ins, reason="serialize repeat")
        xt_sb = constp.tile([C, N], BF16)
        for r in range(NT // YR):
            # quarter chunks so early y rounds don't wait on the full load
            cs_ = slice(r * YR * P, (r + 1) * YR * P)
            nc.gpsimd.dma_start(xt_sb[:, cs_], xT[:, cs_])
        xto_sb = constp.tile([C, JW], BF16)
        nc.gpsimd.dma_start(xto_sb[:], xTo)
        b_row = constp.tile([C, 1], F32)
        nc.gpsimd.dma_start(b_row[:], b_d)
        adiag_row = constp.tile([1, JW], F32)
        nc.gpsimd.dma_start(adiag_row[:], adiag)
        ones2 = constp.tile([P, 2, C], F8)   # cs DoubleRow stationary
        nc.vector.memset(ones2[:], 1.0)
        ones1 = constp.tile([P, 1], F8)      # fill stationary
        nc.vector.memset(ones1[:], 1.0)
        ones1f = constp.tile([1, C], F32)    # r3 broadcast stationary
        nc.vector.memset(ones1f[:], 1.0)
        onesD = constp.tile([1, C], F32)     # dis broadcast stationary (1/ZS)
        nc.vector.memset(onesD[:], 1.0 / ZS)

        slab = slabp.tile([P, NPC, NT, PW], F8)   # 64 KB/partition
        y_sb = yp.tile([P, NT, C], BF16)
        z_sb = zp.tile([P, NT, C], F8)
        # gathered rdegS, stored [128, piece, u, core-slot(pad 8->16)]: the
        # pad keeps (u, c) unmergeable so the transposed DMA balances in 3D
        dis_col = constp.tile([P, NPC * 2 * 16], F32)
        rdeg = rowsp.tile([1, JW], F32, tag="rdeg")
        dis_row = rowsp.tile([1, JW], F32, tag="dis")
        r3_row = rowsp.tile([1, JW], F32, tag="r3")
        rdegS_row = rowsp.tile([1, JW], F32, tag="rdegS")

        y_scr = psyr.tile([P, YR, C], F32)        # 1 bank, 4 rounds
        cs = pscs.tile([C, 2, PW], F32, tag="cs")  # 1 bank, 2 piece slots
        s_ps = [
            pss.tile([C, PW], F32, tag=f"s{q}", name=f"s_ps{q}")
            for q in range(NPC)
        ]
        bc2 = psbc.tile([C, 2, PW], F32, tag="bc2")   # 1 bank, per piece
        fill_ps = bc2[:, 1]                            # pre-ph2 fills only
        yt_scr = psyt.tile([C, JW // 2], F32, tag="yt")  # 1 bank, 2 rounds
        bq_sb = bcp.tile([C, 2, JW], F32)
        yt_sb = bcp.tile([C, JW], F32)

        cc_ins = [
            dramp.tile([PW], F32, name=f"cc_in{p}") for p in range(NPC)
        ]
        cc_outs = [
            dramp.tile(
                [n_devices * PW], F32,
                addr_space="Shared" if use_cc else "Local",
                name=f"cc_out{p}",
            )
            for p in range(NPC)
        ]

        # ---- slab DMA stream: piece-major, contiguous 512KB chunks on the
        # SP (sync) HWDGE queue; ACT queue holds only the tiny dis loads. ----
        last_chunk = None
        for p in range(NPC):
            for ch in range(NCH):
                last_chunk = nc.sync.dma_start(
                    slab[:, p, ch * TPC:(ch + 1) * TPC, :],
                    adjs[p, ch],
                )
                if prev_inst is not None and p == 0 and ch == 0:
                    from concourse.tile_rust import add_dep_helper
                    add_dep_helper(
                        last_chunk.ins, prev_inst.ins, reason="serialize"
                    )

        if variant == "dmaonly":
            v = outp.tile([C, JW], F32)
            nc.vector.memset(v[:], 0.0)
            from concourse.tile_rust import add_dep_helper
            last = nc.sync.dma_start(out_d[:], v[:])
            add_dep_helper(last.ins, last_chunk.ins, reason="gate on slab")
            return last

        def emit_cs(p, t):
            # column-sum DoubleRow matmul over tile pair (t, t+1) of piece p;
            # all 32 output rows are identical (all-ones stationary)
            nc.tensor.matmul(
                cs[:, p % 2, :],
                ones2[:],
                slab[:, p, t:t + 2, :],
                start=(t == 0),
                stop=(t == NT - 2),
                skip_group_check=True,
                perf_mode=PM.DoubleRow,
            )

        def emit_y(r):
            # y = x @ W for tiles [r*YR, (r+1)*YR) through the 1-bank PSUM
            # scratch, then a DVE copy out to SBUF bf16
            for tt in range(YR):
                t = r * YR + tt
                nc.tensor.matmul(
                    y_scr[:, tt, :],
                    xt_sb[:, t * P:(t + 1) * P],
                    w_bf[:],
                    start=True,
                    stop=True,
                )
            nc.vector.tensor_copy(
                y_sb[:, r * YR:(r + 1) * YR, :], y_scr[:]
            )

        dis4 = dis_col.rearrange("z (pp u c) -> z pp u c", pp=NPC, u=2)

        if variant == "peonly":
            # pure PE throughput: same matmul mix, all reading piece 0's
            # first chunk; no stream dependency, no AllGather
            nc.vector.memset(dis_col[:], 0.5)
            for r in range(NT // YR):
                emit_y(r)
            for c8 in range(n_devices):
                for pp in range(NPC):
                    t0 = c8 * (2 * NPC) + 2 * pp
                    nc.vector.tensor_tensor(
                        z_sb[:, t0:t0 + 2, :], y_sb[:, t0:t0 + 2, :],
                        dis4[:, pp, :, c8].unsqueeze(2).broadcast_to([P, 2, C]),
                        op=Alu.mult,
                    )
            for t in range(0, NT, 2):
                for q in range(NPC):
                    nc.tensor.matmul(
                        cs[:, q % 2, :], ones2[:],
                        slab[:, 0, (t % TPC):(t % TPC) + 2, :],
                        start=(t == 0), stop=(t == NT - 2),
                        skip_group_check=True,
                        perf_mode=PM.DoubleRow,
                    )
            for t in range(0, NT, 2):
                for q in range(NPC):
                    nc.tensor.matmul(
                        s_ps[q][:], z_sb[:, t:t + 2, :],
                        slab[:, 0, (t % TPC):(t % TPC) + 2, :],
                        start=(t == 0), stop=(t == NT - 2),
                        skip_group_check=True,
                        perf_mode=PM.DoubleRow,
                    )
            u = outp.tile([C, JW], F32)
            for q in range(NPC):
                nc.vector.tensor_copy(u[:, q * PW:(q + 1) * PW], s_ps[q][:])
            return nc.sync.dma_start(out_d[:], u[:])

        first_q = {q: True for q in range(NPC)}

        def emit_ph2(t, q, stop):
            nc.tensor.matmul(
                s_ps[q][:],
                z_sb[:, t:t + 2, :],
                slab[:, q, t:t + 2, :],
                start=first_q[q],
                stop=stop,
                skip_group_check=True,
                perf_mode=PM.DoubleRow,
            )
            first_q[q] = False

        def emit_fill(ns):
            # cheap PE keep-warm matmuls reading the first (resident) chunk;
            # target shares the bcast bank (bcasts are emitted later, in the
            # tail, so the framework sees fill->bcast as plain WAW)
            n = max(0, int(ns / (PW * COL_NS)))
            for _ in range(n):
                nc.tensor.matmul(
                    fill_ps[0:1],
                    ones1[:, 0:1],
                    slab[:, 0, 0, :],
                    start=True,
                    stop=True,
                    skip_group_check=True,
                )

        def emit_bcast(p):
            # broadcast piece p's dis/r3 row across the 32 channel rows;
            # the dis broadcast also divides out the z fp8 pre-scale ZS
            for slot, ones_st, row in (
                (0, onesD, dis_row), (1, ones1f, r3_row)
            ):
                nc.tensor.matmul(
                    bc2[:, slot, :],
                    ones_st[:],
                    row[:, p * PW:(p + 1) * PW],
                    start=True,
                    stop=True,
                    skip_group_check=True,
                )
            nc.vector.tensor_copy(
                bq_sb[:, :, p * PW:(p + 1) * PW], bc2[:]
            )

        def emit_deg_ag(p):
            # degree math on partition row 0, piece slice of the flat rows.
            # DVE-only up to the collective so a slow AllGather can never
            # block the next piece's degree chain.
            r = (slice(0, 1), slice(p * PW, (p + 1) * PW))
            csr = cs[0:1, p % 2, :]
            nc.vector.tensor_sub(rdeg[r], csr, adiag_row[r])
            nc.vector.tensor_scalar(
                rdeg[r], rdeg[r], 2.0, 1.0, op0=Alu.mult, op1=Alu.add
            )
            nc.vector.reciprocal(rdeg[r], rdeg[r])
            nc.vector.tensor_scalar(
                r3_row[r], adiag_row[r], -1.0, 0.5,
                op0=Alu.mult, op1=Alu.add,
            )
            nc.vector.tensor_mul(r3_row[r], r3_row[r], rdeg[r])
            # gather ZS^2 * rdeg; consumers sqrt it into ZS * dis.  The
            # chain up to the collective is DVE-only so a slow AllGather
            # can never delay the next piece's degree math.
            nc.vector.tensor_scalar(
                rdegS_row[r], rdeg[r], ZS * ZS, 0.0,
                op0=Alu.mult, op1=Alu.add,
            )
            # local dis for the (tail) broadcasts / epilogue, on ACT
            nc.scalar.sqrt(dis_row[r], rdeg[r])
            nc.gpsimd.dma_start(cc_ins[p][:], rdegS_row[r])
            if use_cc:
                nc.gpsimd.collective_compute(
                    "AllGather",
                    Alu.bypass,
                    replica_groups=[list(range(n_devices))],
                    ins=[cc_ins[p].opt()],
                    outs=[cc_outs[p].opt()],
                )
            else:
                for rr in range(n_devices):
                    nc.gpsimd.dma_start(
                        cc_outs[p][rr * PW:(rr + 1) * PW], cc_ins[p][:]
                    )

        def emit_consume(p):
            # AG consumer, all on the ACT queue: transposed gather load,
            # sqrt into ZS*dis, then z = y * (ZS*dis) per core-slot.
            # Emitted only after every piece's degree chain + AllGather
            # issue, so a stalled consumer cannot delay any AllGather.
            srcv = cc_outs[p].rearrange(
                "(c u q) -> u q c", c=n_devices, q=P
            )
            for uu in range(2):
                nc.scalar.dma_start(dis4[:, p, uu, 0:n_devices], srcv[uu])
            nc.scalar.sqrt(
                dis4[:, p, :, 0:n_devices], dis4[:, p, :, 0:n_devices]
            )
            for c8 in range(n_devices):
                t0 = c8 * (2 * NPC) + 2 * p
                for uu in range(2):
                    nc.scalar.activation(
                        z_sb[:, t0 + uu, :],
                        y_sb[:, t0 + uu, :],
                        AF.Copy,
                        scale=dis4[:, p, uu, c8:c8 + 1],
                    )

        # ---- emission: cs backlog (stream-paced) + early y; each piece's
        # AllGather fires as soon as its column sums exist ----
        for p in range(NPC):
            for ch in range(NCH):
                s = p * NCH + ch
                for tt in range(0, TPC, 2):
                    emit_cs(p, ch * TPC + tt)
                if s in (1, 2):
                    emit_y(2 * (s - 1))
                    emit_y(2 * (s - 1) + 1)
                if ch == NCH - 1:
                    emit_deg_ag(p)

        # AG consumers (ACT queue only; ordered, each gated on its AG)
        for p in range(NPC):
            emit_consume(p)

        # optional PE keep-warm while the first AllGather is in flight
        emit_fill(TAILFILL)

        # ---- phase-2, z-piece-major: piece p's tiles unlock when AG p
        # lands; all four PSUM groups accumulate across the whole phase ----
        zp_tiles = [[] for _ in range(NPC)]
        for t in range(0, NT, 2):
            zp_tiles[piece_of(t)].append(t)
        for zp in range(NPC):
            for i, t in enumerate(zp_tiles[zp]):
                for q in range(NPC):
                    emit_ph2(t, q, stop=(zp == NPC - 1 and
                                         i == len(zp_tiles[zp]) - 1))

        # ---- tail: broadcasts + yT, then epilogue ----
        for p in range(NPC):
            emit_bcast(p)
        for g in range(2):
            nc.tensor.matmul(
                yt_scr[:],
                w_bf[:],
                xto_sb[:, g * (JW // 2):(g + 1) * (JW // 2)],
                start=True,
                stop=True,
            )
            nc.vector.tensor_copy(
                yt_sb[:, g * (JW // 2):(g + 1) * (JW // 2)], yt_scr[:]
            )

        # ---- epilogue: out = tanh(2*(dis*s + r3*yT) + b), flat layout ----
        u = outp.tile([C, JW], F32)
        v = outp.tile([C, JW], F32)
        for q in range(NPC):
            nc.vector.tensor_mul(
                u[:, q * PW:(q + 1) * PW], s_ps[q][:],
                bq_sb[:, 0, q * PW:(q + 1) * PW],
            )
        nc.vector.tensor_mul(v[:], yt_sb[:], bq_sb[:, 1])
        nc.vector.tensor_add(u[:], u[:], v[:])
        nc.scalar.activation(v[:], u[:], AF.Tanh, bias=b_row[:], scale=2.0)
        return nc.sync.dma_start(out_d[:], v[:])


_NC_CACHE = None


def _get_nc():
    global _NC_CACHE
    if _NC_CACHE is None:
        _NC_CACHE = build_kernel()
    return _NC_CACHE


def _pack_adj(a):
    # [8192, 1024] f32 -> fp8 packed [piece, chunk, partition, tile, cols]
    a = a.astype(NP_F8).reshape(NCH, TPC, P, NPC, PW)
    return np.ascontiguousarray(a.transpose(3, 0, 2, 1, 4))


def make_in_maps(x, adj, W, b):
    x = np.ascontiguousarray(np.asarray(x, dtype=np.float32))
    adj = np.ascontiguousarray(np.asarray(adj, dtype=np.float32))
    W = np.ascontiguousarray(np.asarray(W, dtype=np.float32))
    b = np.ascontiguousarray(np.asarray(b, dtype=np.float32))

    xT = np.ascontiguousarray(x.T)
    xT_bf = np.ascontiguousarray(xT.astype(BF))
    W_bf = W.astype(BF)
    diag = np.ascontiguousarray(np.diagonal(adj)).astype(np.float32)

    in_maps = []
    for c in range(NCORES):
        js = slice(c * JW, (c + 1) * JW)
        in_maps.append(
            {
                "adjs": _pack_adj(adj[:, js]),
                "xT": xT_bf,
                "xTo": np.ascontiguousarray(xT_bf[:, js]),
                "Wbf": W_bf,
                "b": b,
                "adiag": np.ascontiguousarray(diag[js]),
            }
        )
    return in_maps


def kernel(x, adj, W, b, **run_kwargs):
    nc = _get_nc()
    in_maps = make_in_maps(x, adj, W, b)

    res = run_bass_kernel_spmd(
        nc, in_maps, core_ids=list(range(NCORES)), **run_kwargs
    )
    out = np.concatenate(
        [res.results[c]["out"] for c in range(NCORES)], axis=1
    )
    if run_kwargs:
        return out, res
    return out
